# revision 5
# baseline (speedup 1.0000x reference)
"""Distributed Trainium2 kernel for the AnaC2f GNN message-passing problem.

Reference computation (B=16, C=128, H=W=160):
  - per batch: select top-256 score positions, gather their C-dim features
  - merge all batches into one 4096-node graph
  - cosine-similarity graph (threshold 0.6, includes self loops)
  - one GCN layer: D^-1/2 A D^-1/2 X @ W + b
  - scatter updated features back into z, return full [B, C, H, W]

Sharding: data-parallel over batch across 8 NeuronCores (2 batches/core).
Each core streams its z shard to its output shard (the memory-bound part),
computes similarity rows + degrees for its own 512 nodes against the
(replicated) full node set, and the full degree vector is assembled with an
AllGather collective.  Top-k index selection runs on host (cheap, index-only);
all feature compute runs on device.
"""

import sys

sys.path.insert(0, "/opt/trn_rl_repo")

import numpy as np

import concourse.bass as bass
import concourse.tile as tile
from concourse import bacc, mybir
from concourse.bass_utils import run_bass_kernel_spmd

F32 = mybir.dt.float32
ALU = mybir.AluOpType
ACTF = mybir.ActivationFunctionType

B, C, H, W = 16, 128, 160, 160
HW = H * W
S = 256                # selected positions per batch (HW * 0.01)
NCORES = 8
BLOC = B // NCORES     # batches per core
SLOC = BLOC * S        # local nodes per core
N = B * S              # global nodes
NCHUNK = N // 128      # 32 chunks of 128 global nodes
SIM_THRESHOLD = 0.6

_cache = {}


def _build():
    nc = bacc.Bacc("TRN2", target_bir_lowering=False, debug=False)

    z0 = nc.declare_dram_parameter("z0", [C, HW], F32, isOutput=False)
    z1 = nc.declare_dram_parameter("z1", [C, HW], F32, isOutput=False)
    featsT = nc.declare_dram_parameter("featsT", [C, N], F32, isOutput=False)
    ftloc = nc.declare_dram_parameter("ftloc", [C, SLOC], F32, isOutput=False)
    fnm = nc.declare_dram_parameter("fnm", [N, C], F32, isOutput=False)
    Wg = nc.declare_dram_parameter("Wg", [C, C], F32, isOutput=False)
    bg = nc.declare_dram_parameter("bg", [C, 1], F32, isOutput=False)

    out0 = nc.declare_dram_parameter("out0", [C, HW], F32, isOutput=True)
    out1 = nc.declare_dram_parameter("out1", [C, HW], F32, isOutput=True)
    updT_out = nc.declare_dram_parameter("updT", [C, SLOC], F32, isOutput=True)

    cc_in = nc.dram_tensor("cc_in", [SLOC], F32)
    cc_out = nc.dram_tensor("cc_out", [N], F32, addr_space="Shared")
    rn_dram = nc.dram_tensor("rn_dram", [N], F32)
    rnl_dram = nc.dram_tensor("rnl_dram", [SLOC], F32)
    dinvl_dram = nc.dram_tensor("dinvl_dram", [SLOC], F32)

    with tile.TileContext(nc) as tc:
        with (
            tc.tile_pool(name="inp", bufs=1) as inp,
            tc.tile_pool(name="big", bufs=1) as big,
            tc.tile_pool(name="mid", bufs=1) as mid,
            tc.tile_pool(name="small", bufs=1) as small,
            tc.tile_pool(name="ps", bufs=4, space="PSUM") as ps,
            tc.tile_pool(name="psacc", bufs=1, space="PSUM") as psacc,
        ):
            # ---- bulk z -> out copies (the memory-bound stream), SWDGE queue
            nc.gpsimd.dma_start(out=out0[:], in_=z0[:])
            nc.gpsimd.dma_start(out=out1[:], in_=z1[:])

            # ---- load GCN inputs (HWDGE queue)
            featsT_t = inp.tile([C, N], F32)
            nc.sync.dma_start(out=featsT_t[:], in_=featsT[:])
            fnm_t = inp.tile([128, NCHUNK, C], F32)
            nc.sync.dma_start(
                out=fnm_t[:], in_=fnm[:].rearrange("(g p) c -> p g c", p=128)
            )
            ftloc_t = inp.tile([C, SLOC], F32)
            nc.sync.dma_start(out=ftloc_t[:], in_=ftloc[:])
            W_t = inp.tile([C, C], F32)
            nc.sync.dma_start(out=W_t[:], in_=Wg[:])
            b_t = inp.tile([C, 1], F32)
            nc.sync.dma_start(out=b_t[:], in_=bg[:])
            ones_t = inp.tile([128, 1], F32)
            nc.vector.memset(ones_t[:], 1.0)

            # ---- node norms (all nodes, C-major): ss = sum_c feats^2
            sq_t = mid.tile([C, N], F32)
            nc.vector.tensor_tensor(sq_t[:], featsT_t[:], featsT_t[:], op=ALU.mult)
            ssr = small.tile([1, N], F32)
            for j in range(N // 512):
                ss_ps = ps.tile([1, 512], F32, tag="mm")
                nc.tensor.matmul(
                    ss_ps[:], ones_t[:], sq_t[:, j * 512 : (j + 1) * 512],
                    start=True, stop=True,
                )
                nc.vector.tensor_scalar_max(ssr[:, j * 512 : (j + 1) * 512], ss_ps[:], 1e-24)
            sroot = small.tile([1, N], F32)
            nc.scalar.activation(sroot[:], ssr[:], ACTF.Sqrt)
            rn_row = small.tile([1, N], F32)
            nc.vector.reciprocal(rn_row[:], sroot[:])

            # local norms (same arithmetic on the local slice input)
            sql_t = small.tile([C, SLOC], F32)
            nc.vector.tensor_tensor(sql_t[:], ftloc_t[:], ftloc_t[:], op=ALU.mult)
            ssl = small.tile([1, SLOC], F32)
            ssl_ps = ps.tile([1, 512], F32, tag="mm")
            nc.tensor.matmul(ssl_ps[:], ones_t[:], sql_t[:], start=True, stop=True)
            nc.vector.tensor_scalar_max(ssl[:], ssl_ps[:], 1e-24)
            srootl = small.tile([1, SLOC], F32)
            nc.scalar.activation(srootl[:], ssl[:], ACTF.Sqrt)
            rnl_row = small.tile([1, SLOC], F32)
            nc.vector.reciprocal(rnl_row[:], srootl[:])

            # broadcast rn across partitions via DRAM bounce
            nc.gpsimd.dma_start(out=rn_dram[:], in_=rn_row[:])
            rn_b = mid.tile([C, N], F32, tag="rn_b")
            nc.gpsimd.dma_start(out=rn_b[:], in_=rn_dram[None, :].to_broadcast((C, N)))
            nc.gpsimd.dma_start(out=rnl_dram[:], in_=rnl_row[:])
            rnl_b = small.tile([C, SLOC], F32)
            nc.gpsimd.dma_start(
                out=rnl_b[:], in_=rnl_dram[None, :].to_broadcast((C, SLOC))
            )

            # normalize: nfT (in place over featsT_t), nfl
            nc.vector.tensor_tensor(featsT_t[:], featsT_t[:], rn_b[:], op=ALU.mult)
            nfl_t = small.tile([C, SLOC], F32)
            nc.vector.tensor_tensor(nfl_t[:], ftloc_t[:], rnl_b[:], op=ALU.mult)

            # ---- similarity rows (transposed): adjT[g*128+p, i] for local i
            adjT_t = big.tile([128, NCHUNK, 512], F32)
            for g in range(NCHUNK):
                sim_ps = ps.tile([128, 512], F32, tag="mm")
                nc.tensor.matmul(
                    sim_ps[:],
                    featsT_t[:, g * 128 : (g + 1) * 128],
                    nfl_t[:],
                    start=True, stop=True,
                )
                nc.vector.tensor_scalar(
                    adjT_t[:, g, :], sim_ps[:], SIM_THRESHOLD, None, op0=ALU.is_gt
                )

            # ---- degrees of local nodes: deg_i = sum_j adjT[j, i]
            deg_ps = psacc.tile([1, 512], F32, tag="deg")
            for g in range(NCHUNK):
                nc.tensor.matmul(
                    deg_ps[:], ones_t[:], adjT_t[:, g, :],
                    start=(g == 0), stop=(g == NCHUNK - 1),
                )
            deg_row = small.tile([1, SLOC], F32)
            nc.vector.tensor_copy(deg_row[:], deg_ps[:])

            # ---- AllGather degrees -> full degree vector
            nc.sync.dma_start(out=cc_in[:], in_=deg_row[:])
            nc.gpsimd.collective_compute(
                "AllGather",
                ALU.bypass,
                replica_groups=[list(range(NCORES))],
                ins=[cc_in[:]],
                outs=[cc_out[:]],
            )

            # local dinv = 1/sqrt(max(deg,1)) -> row + broadcast
            dl0 = small.tile([1, SLOC], F32)
            nc.vector.tensor_scalar_max(dl0[:], deg_row[:], 1.0)
            dl1 = small.tile([1, SLOC], F32)
            nc.scalar.activation(dl1[:], dl0[:], ACTF.Sqrt)
            dinvl_row = small.tile([1, SLOC], F32)
            nc.vector.reciprocal(dinvl_row[:], dl1[:])
            nc.gpsimd.dma_start(out=dinvl_dram[:], in_=dinvl_row[:])
            dinvl_b = small.tile([C, SLOC], F32)
            nc.gpsimd.dma_start(
                out=dinvl_b[:], in_=dinvl_dram[None, :].to_broadcast((C, SLOC))
            )

            # global dinv in node-major layout [128, NCHUNK]
            dgm_t = small.tile([128, 128], F32)
            nc.vector.memset(dgm_t[:], 1.0)
            nc.sync.dma_start(
                out=dgm_t[:NCHUNK, :],
                in_=cc_out[:].rearrange("(g p) -> g p", g=NCHUNK),
            )
            dgm_T = small.tile([128, 128], F32)
            nc.vector.transpose(dgm_T[:], dgm_t[:])
            deg_nm = small.tile([128, NCHUNK], F32)
            nc.vector.tensor_copy(deg_nm[:], dgm_T[:, :NCHUNK])
            nc.vector.tensor_scalar_max(deg_nm[:], deg_nm[:], 1.0)
            dnm1 = small.tile([128, NCHUNK], F32)
            nc.scalar.activation(dnm1[:], deg_nm[:], ACTF.Sqrt)
            dinv_nm = small.tile([128, NCHUNK], F32)
            nc.vector.reciprocal(dinv_nm[:], dnm1[:])

            # df = dinv_j * feats_j, node-major (in place over fnm_t)
            for g in range(NCHUNK):
                nc.vector.tensor_scalar(
                    fnm_t[:, g, :], fnm_t[:, g, :], dinv_nm[:, g : g + 1], None,
                    op0=ALU.mult,
                )

            # ---- aggregation: yT[c, i] = sum_j df[j, c] * adjT[j, i]
            yT_ps = psacc.tile([C, 512], F32, tag="yT")
            for g in range(NCHUNK):
                nc.tensor.matmul(
                    yT_ps[:], fnm_t[:, g, :], adjT_t[:, g, :],
                    start=(g == 0), stop=(g == NCHUNK - 1),
                )
            yT_sb = small.tile([C, SLOC], F32)
            nc.vector.tensor_copy(yT_sb[:], yT_ps[:])

            # ---- updated^T = dinv_i * (W^T @ yT) + b
            uT_ps = psacc.tile([C, 512], F32, tag="uT")
            nc.tensor.matmul(uT_ps[:], W_t[:], yT_sb[:], start=True, stop=True)
            updT_sb = small.tile([C, SLOC], F32)
            nc.vector.tensor_tensor(updT_sb[:], uT_ps[:], dinvl_b[:], op=ALU.mult)
            nc.vector.tensor_scalar(
                updT_sb[:], updT_sb[:], b_t[:, 0:1], None, op0=ALU.add
            )
            nc.sync.dma_start(out=updT_out[:], in_=updT_sb[:])

    nc.compile()
    return nc


def _get_nc():
    if "nc" not in _cache:
        _cache["nc"] = _build()
    return _cache["nc"]


def kernel(z, score, W_gcn, b_gcn):
    z = np.ascontiguousarray(z, dtype=np.float32)
    score = np.ascontiguousarray(score, dtype=np.float32)
    W_gcn = np.ascontiguousarray(W_gcn, dtype=np.float32)
    b_gcn = np.ascontiguousarray(b_gcn, dtype=np.float32)

    flat_z = z.reshape(B, C, HW)
    flat_score = score.reshape(B, HW)

    # host: top-k index selection (order irrelevant: the GCN is
    # permutation-equivariant and the scatter uses the same ordering)
    top_idx = np.argpartition(-flat_score, S - 1, axis=1)[:, :S].astype(np.int32)

    # host: gather selected features
    feats = np.take_along_axis(flat_z, top_idx[:, None, :], axis=2)  # [B, C, S]
    featsT_all = np.ascontiguousarray(
        feats.transpose(1, 0, 2).reshape(C, N)
    )  # node n = b*S + s
    fnm_all = np.ascontiguousarray(featsT_all.T)  # [N, C]
    bg_col = b_gcn.reshape(C, 1)

    in_maps = []
    for i in range(NCORES):
        in_maps.append(
            {
                "z0": flat_z[2 * i],
                "z1": flat_z[2 * i + 1],
                "featsT": featsT_all,
                "ftloc": np.ascontiguousarray(
                    featsT_all[:, i * SLOC : (i + 1) * SLOC]
                ),
                "fnm": fnm_all,
                "Wg": W_gcn,
                "bg": bg_col,
            }
        )

    nc = _get_nc()
    res = run_bass_kernel_spmd(nc, in_maps, list(range(NCORES))).results

    out = np.empty((B, C, HW), dtype=np.float32)
    for i in range(NCORES):
        out[2 * i] = res[i]["out0"]
        out[2 * i + 1] = res[i]["out1"]
        updT = res[i]["updT"]  # [C, SLOC]
        for bl in range(BLOC):
            b = 2 * i + bl
            out[b][:, top_idx[b]] = updT[:, bl * S : (bl + 1) * S]
    return out.reshape(B, C, H, W)


# revision 7
# speedup vs baseline: 1.4685x; 1.4685x over previous
"""Distributed Trainium2 kernel for the AnaC2f GNN message-passing problem.

Reference computation (B=16, C=128, H=W=160):
  - per batch: select top-256 score positions, gather their C-dim features
  - merge all batches into one 4096-node graph
  - cosine-similarity graph (threshold 0.6, includes self loops)
  - one GCN layer: D^-1/2 A D^-1/2 X @ W + b
  - scatter updated features back into z, return full [B, C, H, W]

Sharding: data-parallel over batch across 8 NeuronCores (2 batches/core).
Each core streams its z shard to its output shard (the memory-bound part),
computes similarity rows + degrees for its own 512 nodes against the
(replicated) full node set, and the full degree vector is assembled with an
AllGather collective.  Top-k index selection runs on host (cheap, index-only);
all feature compute runs on device.
"""

import sys

sys.path.insert(0, "/opt/trn_rl_repo")

import numpy as np

import concourse.bass as bass
import concourse.tile as tile
from concourse import bacc, mybir
from concourse.bass_utils import run_bass_kernel_spmd

F32 = mybir.dt.float32
BF16 = mybir.dt.bfloat16
ALU = mybir.AluOpType
ACTF = mybir.ActivationFunctionType

B, C, H, W = 16, 128, 160, 160
HW = H * W
S = 256                # selected positions per batch (HW * 0.01)
NCORES = 8
BLOC = B // NCORES     # batches per core
SLOC = BLOC * S        # local nodes per core
N = B * S              # global nodes
NCHUNK = N // 128      # 32 chunks of 128 global nodes
SIM_THRESHOLD = 0.6

_cache = {}


def _build():
    nc = bacc.Bacc("TRN2", target_bir_lowering=False, debug=False)

    z0 = nc.declare_dram_parameter("z0", [C, HW], F32, isOutput=False)
    z1 = nc.declare_dram_parameter("z1", [C, HW], F32, isOutput=False)
    featsT = nc.declare_dram_parameter("featsT", [C, N], F32, isOutput=False)
    ftloc = nc.declare_dram_parameter("ftloc", [C, SLOC], F32, isOutput=False)
    fnm = nc.declare_dram_parameter("fnm", [C, N], F32, isOutput=False)  # [p, g*128+c] pre-swizzled
    Wg = nc.declare_dram_parameter("Wg", [C, C], F32, isOutput=False)
    bg = nc.declare_dram_parameter("bg", [C, 1], F32, isOutput=False)

    out0 = nc.declare_dram_parameter("out0", [C, HW], F32, isOutput=True)
    out1 = nc.declare_dram_parameter("out1", [C, HW], F32, isOutput=True)
    updT_out = nc.declare_dram_parameter("updT", [C, SLOC], F32, isOutput=True)

    cc_in = nc.dram_tensor("cc_in", [SLOC], F32)
    cc_out = nc.dram_tensor("cc_out", [N], F32, addr_space="Shared")
    rn_dram = nc.dram_tensor("rn_dram", [N], F32)
    ss_dram = nc.dram_tensor("ss_dram", [N], F32)
    rnl_dram = nc.dram_tensor("rnl_dram", [SLOC], F32)
    dinvl_dram = nc.dram_tensor("dinvl_dram", [SLOC], F32)

    with tile.TileContext(nc) as tc:
        with (
            tc.tile_pool(name="inp", bufs=1) as inp,
            tc.tile_pool(name="big", bufs=1) as big,
            tc.tile_pool(name="mid", bufs=1) as mid,
            tc.tile_pool(name="small", bufs=1) as small,
            tc.tile_pool(name="ps", bufs=4, space="PSUM") as ps,
            tc.tile_pool(name="psacc", bufs=1, space="PSUM") as psacc,
        ):
            # ---- bulk z -> out copies (the memory-bound stream), SWDGE queue
            nc.gpsimd.dma_start(out=out0[:], in_=z0[:])
            nc.gpsimd.dma_start(out=out1[:], in_=z1[:])

            # ---- load GCN inputs (HWDGE queue)
            featsT_t = inp.tile([C, N], F32)
            nc.sync.dma_start(out=featsT_t[:], in_=featsT[:])
            fnm_t = inp.tile([128, NCHUNK, C], F32)
            nc.sync.dma_start(out=fnm_t[:], in_=fnm[:])
            ftloc_t = inp.tile([C, SLOC], F32)
            nc.sync.dma_start(out=ftloc_t[:], in_=ftloc[:])
            W_t = inp.tile([C, C], F32)
            nc.sync.dma_start(out=W_t[:], in_=Wg[:])
            b_t = inp.tile([C, 1], F32)
            nc.sync.dma_start(out=b_t[:], in_=bg[:])
            ones_t = inp.tile([128, 1], F32)
            nc.vector.memset(ones_t[:], 1.0)
            ones_bf = inp.tile([128, 1], BF16)
            nc.vector.memset(ones_bf[:], 1.0)

            # ---- node norms (all nodes, C-major): ss = sum_c feats^2
            sq_t = mid.tile([C, N], F32)
            nc.vector.tensor_tensor(sq_t[:], featsT_t[:], featsT_t[:], op=ALU.mult)
            ssr = small.tile([1, N], F32)
            for j in range(N // 512):
                ss_ps = ps.tile([1, 512], F32, tag="mm")
                nc.tensor.matmul(
                    ss_ps[:], ones_t[:], sq_t[:, j * 512 : (j + 1) * 512],
                    start=True, stop=True,
                )
                nc.vector.tensor_scalar_max(ssr[:, j * 512 : (j + 1) * 512], ss_ps[:], 1e-24)
            nc.gpsimd.dma_start(out=ss_dram[:], in_=ssr[:])
            ss_w = small.tile([NCHUNK, 128], F32)
            nc.gpsimd.dma_start(out=ss_w[:], in_=ss_dram[:].rearrange("(g p) -> g p", g=NCHUNK))
            sroot_w = small.tile([NCHUNK, 128], F32)
            nc.scalar.activation(sroot_w[:], ss_w[:], ACTF.Sqrt)
            rn_w = small.tile([NCHUNK, 128], F32)
            nc.vector.reciprocal(rn_w[:], sroot_w[:])

            # local norms (same arithmetic on the local slice input)
            sql_t = small.tile([C, SLOC], F32)
            nc.vector.tensor_tensor(sql_t[:], ftloc_t[:], ftloc_t[:], op=ALU.mult)
            ssl = small.tile([1, SLOC], F32)
            ssl_ps = ps.tile([1, 512], F32, tag="mm")
            nc.tensor.matmul(ssl_ps[:], ones_t[:], sql_t[:], start=True, stop=True)
            nc.vector.tensor_scalar_max(ssl[:], ssl_ps[:], 1e-24)
            srootl = small.tile([1, SLOC], F32)
            nc.scalar.activation(srootl[:], ssl[:], ACTF.Sqrt)
            rnl_row = small.tile([1, SLOC], F32)
            nc.vector.reciprocal(rnl_row[:], srootl[:])

            # broadcast rn across partitions via DRAM bounce
            nc.gpsimd.dma_start(
                out=rn_dram[:].rearrange("(g p) -> g p", g=NCHUNK), in_=rn_w[:]
            )
            rn_b = mid.tile([C, N], F32, tag="rn_b")
            nc.gpsimd.dma_start(out=rn_b[:], in_=rn_dram[None, :].to_broadcast((C, N)))
            nc.gpsimd.dma_start(out=rnl_dram[:], in_=rnl_row[:])
            rnl_b = small.tile([C, SLOC], F32)
            nc.gpsimd.dma_start(
                out=rnl_b[:], in_=rnl_dram[None, :].to_broadcast((C, SLOC))
            )

            # normalize (bf16 outputs for the PE): nfT, nfl
            nfT_bf = mid.tile([C, N], BF16, tag="nfT_bf")
            nc.vector.tensor_tensor(nfT_bf[:], featsT_t[:], rn_b[:], op=ALU.mult)
            nfl_bf = small.tile([C, SLOC], BF16)
            nc.vector.tensor_tensor(nfl_bf[:], ftloc_t[:], rnl_b[:], op=ALU.mult)

            # ---- similarity rows (transposed): adjT[g*128+p, i] for local i
            adjT_t = big.tile([128, NCHUNK, 512], BF16)
            for g in range(NCHUNK):
                sim_ps = ps.tile([128, 512], F32, tag="mm")
                nc.tensor.matmul(
                    sim_ps[:],
                    nfT_bf[:, g * 128 : (g + 1) * 128],
                    nfl_bf[:],
                    start=True, stop=True,
                )
                nc.vector.tensor_scalar(
                    adjT_t[:, g, :], sim_ps[:], SIM_THRESHOLD, None, op0=ALU.is_gt
                )

            # ---- degrees of local nodes: deg_i = sum_j adjT[j, i]
            deg_ps = psacc.tile([1, 512], F32, tag="deg")
            for g in range(NCHUNK):
                nc.tensor.matmul(
                    deg_ps[:], ones_bf[:], adjT_t[:, g, :],
                    start=(g == 0), stop=(g == NCHUNK - 1),
                )
            deg_row = small.tile([1, SLOC], F32)
            nc.vector.tensor_copy(deg_row[:], deg_ps[:])

            # ---- AllGather degrees -> full degree vector
            nc.sync.dma_start(out=cc_in[:], in_=deg_row[:])
            nc.gpsimd.collective_compute(
                "AllGather",
                ALU.bypass,
                replica_groups=[list(range(NCORES))],
                ins=[cc_in[:]],
                outs=[cc_out[:]],
            )

            # local dinv = 1/sqrt(max(deg,1)) -> row + broadcast
            dl0 = small.tile([1, SLOC], F32)
            nc.vector.tensor_scalar_max(dl0[:], deg_row[:], 1.0)
            dl1 = small.tile([1, SLOC], F32)
            nc.scalar.activation(dl1[:], dl0[:], ACTF.Sqrt)
            dinvl_row = small.tile([1, SLOC], F32)
            nc.vector.reciprocal(dinvl_row[:], dl1[:])
            nc.gpsimd.dma_start(out=dinvl_dram[:], in_=dinvl_row[:])
            dinvl_b = small.tile([C, SLOC], F32)
            nc.gpsimd.dma_start(
                out=dinvl_b[:], in_=dinvl_dram[None, :].to_broadcast((C, SLOC))
            )

            # global dinv in node-major layout [128, NCHUNK]
            dgm_t = small.tile([128, 128], F32)
            nc.vector.memset(dgm_t[:], 1.0)
            nc.sync.dma_start(
                out=dgm_t[:NCHUNK, :],
                in_=cc_out[:].rearrange("(g p) -> g p", g=NCHUNK),
            )
            dgm_T = small.tile([128, 128], F32)
            nc.vector.transpose(dgm_T[:], dgm_t[:])
            deg_nm = small.tile([128, NCHUNK], F32)
            nc.vector.tensor_copy(deg_nm[:], dgm_T[:, :NCHUNK])
            nc.vector.tensor_scalar_max(deg_nm[:], deg_nm[:], 1.0)
            dnm1 = small.tile([128, NCHUNK], F32)
            nc.scalar.activation(dnm1[:], deg_nm[:], ACTF.Sqrt)
            dinv_nm = small.tile([128, NCHUNK], F32)
            nc.vector.reciprocal(dinv_nm[:], dnm1[:])

            # df = dinv_j * feats_j, node-major, cast to bf16 for the PE
            df_bf = mid.tile([128, NCHUNK, C], BF16, tag="df_bf")
            for g in range(NCHUNK):
                nc.vector.tensor_scalar(
                    df_bf[:, g, :], fnm_t[:, g, :], dinv_nm[:, g : g + 1], None,
                    op0=ALU.mult,
                )

            # ---- aggregation: yT[c, i] = sum_j df[j, c] * adjT[j, i]
            yT_ps = psacc.tile([C, 512], F32, tag="yT")
            for g in range(NCHUNK):
                nc.tensor.matmul(
                    yT_ps[:], df_bf[:, g, :], adjT_t[:, g, :],
                    start=(g == 0), stop=(g == NCHUNK - 1),
                )
            yT_sb = small.tile([C, SLOC], F32)
            nc.vector.tensor_copy(yT_sb[:], yT_ps[:])

            # ---- updated^T = dinv_i * (W^T @ yT) + b
            uT_ps = psacc.tile([C, 512], F32, tag="uT")
            nc.tensor.matmul(uT_ps[:], W_t[:], yT_sb[:], start=True, stop=True)
            updT_sb = small.tile([C, SLOC], F32)
            nc.vector.tensor_tensor(updT_sb[:], uT_ps[:], dinvl_b[:], op=ALU.mult)
            nc.vector.tensor_scalar(
                updT_sb[:], updT_sb[:], b_t[:, 0:1], None, op0=ALU.add
            )
            nc.sync.dma_start(out=updT_out[:], in_=updT_sb[:])

    nc.compile()
    return nc


def _get_nc():
    if "nc" not in _cache:
        _cache["nc"] = _build()
    return _cache["nc"]


def kernel(z, score, W_gcn, b_gcn):
    z = np.ascontiguousarray(z, dtype=np.float32)
    score = np.ascontiguousarray(score, dtype=np.float32)
    W_gcn = np.ascontiguousarray(W_gcn, dtype=np.float32)
    b_gcn = np.ascontiguousarray(b_gcn, dtype=np.float32)

    flat_z = z.reshape(B, C, HW)
    flat_score = score.reshape(B, HW)

    # host: top-k index selection (order irrelevant: the GCN is
    # permutation-equivariant and the scatter uses the same ordering)
    top_idx = np.argpartition(-flat_score, S - 1, axis=1)[:, :S].astype(np.int32)

    # host: gather selected features
    feats = np.take_along_axis(flat_z, top_idx[:, None, :], axis=2)  # [B, C, S]
    featsT_all = np.ascontiguousarray(
        feats.transpose(1, 0, 2).reshape(C, N)
    )  # node n = b*S + s
    # fnm_dev[p, g*128+c] = feats_nm[g*128+p, c]: node-major chunks pre-swizzled
    # so the device DMA is a plain contiguous [128, N] load
    fnm_all = np.ascontiguousarray(
        featsT_all.reshape(C, NCHUNK, 128).transpose(2, 1, 0).reshape(128, N)
    )
    bg_col = b_gcn.reshape(C, 1)

    in_maps = []
    for i in range(NCORES):
        in_maps.append(
            {
                "z0": flat_z[2 * i],
                "z1": flat_z[2 * i + 1],
                "featsT": featsT_all,
                "ftloc": np.ascontiguousarray(
                    featsT_all[:, i * SLOC : (i + 1) * SLOC]
                ),
                "fnm": fnm_all,
                "Wg": W_gcn,
                "bg": bg_col,
            }
        )

    nc = _get_nc()
    res = run_bass_kernel_spmd(nc, in_maps, list(range(NCORES))).results

    out = np.empty((B, C, HW), dtype=np.float32)
    for i in range(NCORES):
        out[2 * i] = res[i]["out0"]
        out[2 * i + 1] = res[i]["out1"]
        updT = res[i]["updT"]  # [C, SLOC]
        for bl in range(BLOC):
            b = 2 * i + bl
            out[b][:, top_idx[b]] = updT[:, bl * S : (bl + 1) * S]
    return out.reshape(B, C, H, W)
